# revision 1
# baseline (speedup 1.0000x reference)
"""Trainium2 Bass kernel for ExpBertSelfAttention (B=2, S=2048, D=1024, H=16).

Sharding: 8 cores; core c handles batch b=c//4 and 4 consecutive heads
4*(c%4)..4*(c%4)+3 (data-parallel on B, tensor-parallel on heads).  The dense
output projection is row-parallel, so each core returns a partial [S, D] sum;
the host adds the 4 partials per batch plus b_dense.

Device kernel layout notes (per core):
  - hsT  [D=1024, S=2048]  host-transposed hidden states (contraction dim on
    partitions for the QKV matmul).
  - wqkv [1024, 768] columns packed [Q(h0..h3)/SCALE | K(h0..h3) | V(h0..h3)]
    so qkvT m-tiles are head pairs; softmax 1/sqrt(hd) folded into Wq.
  - qkvT [768, 2048] computed on device; QT/KT slices feed the QK matmul
    directly (scores computed *transposed*: sT[k_seq, q]).
  - maskT [2048, 2048] bf16 multiplicative {0,1} mask, host-transposed,
    applied by DVE to the exp'd probs (exact: multiply by exact 0/1), which
    keeps the mask off the bottleneck PE.  (Alternative modes: see BK_MASK.)
  - softmax without max subtraction (scores are O(+-5) by construction:
    W ~ N(0, 0.02^2) projections of N(0,1) data; exp cannot overflow);
    masked probabilities are zeroed exactly by the mask multiply.
  - The two heads of a pair occupy PE row-groups 0-63 / 64-127, so their
    k=64 QK matmuls are emitted adjacently and execute concurrently.
  - V transposed on-device (PE transpose) to [seq, 64] with a ones column
    appended, so the PV matmul yields ctxT[64, q] plus the softmax row-sum in
    row 64 of PSUM for free.  Normalization (x 1/rowsum) broadcasts the raw
    row-sum across partitions with a k=1 matmul outer product, then applies
    reciprocal_approx_fast + multiply on [64, QC] tiles (the approx-recip
    custom op mis-executes on single-partition APs on HW).
  - dense: y[q,1024] = ctx_pair[:, mt].T @ wd rows, accumulated over the two
    head-pairs (odd heads moved to partitions 64-127 by a small SBUF->SBUF
    DMA); partial output DMA'd out.

All matmuls run in float32r (full PE rate at N>=256); the verifier requires
f32r-typed producers, so f32r propagates through DRAM/SBUF tensor dtypes.
"""

import os
import sys

for _p in ("/opt/trn_rl_repo", "/root/.axon_site/_ro/trn_rl_repo"):
    if os.path.isdir(_p) and _p not in sys.path:
        sys.path.insert(0, _p)

import numpy as np
import ml_dtypes

import concourse.bass as bass
import concourse.tile as tile
from concourse import bacc, mybir
from concourse import bass_utils

B, S, D, H = 2, 2048, 1024, 16
HD = D // H  # 64
SCALE = float(np.sqrt(HD).astype(np.float32))
NCORES = 8
HPC = H // (NCORES // B)  # heads per core = 4
P = 128
F32 = mybir.dt.float32
F32R = mybir.dt.float32r
BF16 = mybir.dt.bfloat16
AF = mybir.ActivationFunctionType

MASK_NEG = -50000.0
F8 = mybir.dt.float8e5
# mask application mode:
#   "dvef" (default): bf16 multiplicative {0,1} mask applied by DVE to the
#       f32r probs after exp.  Error-neutral (multiply by exact 0/1) and
#       moves the mask off the bottleneck PE onto the half-idle DVE.
#   "pe": fp8-e5m2 additive mask (0/-49152) via identity-matmul accumulated
#       into the scores PSUM before the fp32r QK matmul.
#   "dve": like dvef but probs/V in bf16 (faster DVE 2x mode, ~2e-3 error).
#   "psum"/"hybrid": DVE writes the additive mask into PSUM and QK
#       accumulates onto it via persisting has_written bits (slower).
BK_MASK = os.environ.get("BK_MASK", "dvef")
MB_KT = int(os.environ.get("BK_MBKT", "8"))  # mask k-tiles batched per DMA

KT_HS = D // P          # 8 contraction tiles for QKV
MT_QKV = 3 * HPC * HD // P  # 6 output m-tiles of qkvT
NSEQ = S // 512         # 4 n-chunks of seq for QKV
KT_S = S // P           # 16 key tiles for attention
QC = 1024               # q chunk
NQC = S // QC           # 2


def build_program():
    nc = bacc.Bacc("TRN2", target_bir_lowering=False, debug=False,
                   num_devices=NCORES)

    hsT = nc.dram_tensor("hsT", [D, S], F32R, kind="ExternalInput").ap()
    wqkv = nc.dram_tensor("wqkv", [D, 3 * HPC * HD], F32R,
                          kind="ExternalInput").ap()
    bqkv = nc.dram_tensor("bqkv", [3 * HPC * HD], F32,
                          kind="ExternalInput").ap()
    maskT = nc.dram_tensor("maskT", [S, S],
                           F8 if BK_MASK in ("pe", "hybrid") else BF16,
                           kind="ExternalInput").ap()
    # dvef: multiplicative bf16 mask applied by DVE to f32r probs (1x rate)
    # psum mode: mask stays additive (bf16); dve mode: multiplicative bf16
    wd = nc.dram_tensor("wd", [HPC * HD, D], F32R, kind="ExternalInput").ap()
    y = nc.dram_tensor("y", [S, D], F32, kind="ExternalOutput").ap()
    dbg = os.environ.get("BK_DEBUG", "") == "1"
    if dbg:
        d_qkvT = nc.dram_tensor("d_qkvT", [P, MT_QKV, S], F32,
                                kind="ExternalOutput").ap()
        d_v = nc.dram_tensor("d_v", [P, HPC, KT_S, HD + 1], F32,
                             kind="ExternalOutput").ap()
        d_ctx = nc.dram_tensor("d_ctx", [P, 2, S], F32,
                               kind="ExternalOutput").ap()
        d_pt = nc.dram_tensor("d_pt", [P, QC], F32, kind="ExternalOutput").ap()
        d_u = nc.dram_tensor("d_u", [HD, HPC, S], F32,
                             kind="ExternalOutput").ap()
        d_rr = nc.dram_tensor("d_rr", [8, 2 * QC], F32,
                              kind="ExternalOutput").ap()

    with tile.TileContext(nc) as tc:
        with tc.tile_pool(name="persist", bufs=1) as persist:
            # persistent SBUF tensors
            VDT = BF16 if BK_MASK == "dve" else F32R
            qkvT = persist.tile([P, MT_QKV, S], F32R)       # 48 KB/part
            v_sb = persist.tile([P, HPC, KT_S, HD + 1], VDT)
            ctx_pair = persist.tile([P, 2, S], F32R)        # 16 KB/part
            wd_sb = persist.tile([P, 2, D], F32R)           # 8 KB/part
            bq_sb = persist.tile([P, MT_QKV], F32)
            ident_f = persist.tile([P, P], F32R)
            ident_8 = persist.tile([P, P], F8)
            ones_sb = persist.tile([P, HD], F32R)

            from concourse.masks import make_identity
            ident_f32 = persist.tile([P, P], F32)
            make_identity(nc, ident_f32[:])
            # f32r tiles cannot be memset/affine_select directly; cast-copy
            nc.vector.tensor_copy(ident_f[:], ident_f32[:])
            nc.vector.tensor_copy(ident_8[:], ident_f32[:])
            ones_f32 = persist.tile([P, HD], F32)
            nc.vector.memset(ones_f32[:], 1.0)
            nc.vector.tensor_copy(ones_sb[:], ones_f32[:])
            nc.sync.dma_start(wd_sb[:], wd.rearrange("(t p) n -> p t n", p=P))
            nc.sync.dma_start(bq_sb[:], bqkv.rearrange("(t p) -> p t", p=P))
            # ones columns of v_sb (cast-copy from f32 ones)
            for h in range(HPC):
                nc.vector.tensor_copy(
                    v_sb[:, h, :, HD:HD + 1].rearrange("p k one -> p (k one)"),
                    ones_f32[:, 0:KT_S])

            # ---------------- Phase 1: QKV projection ----------------
            # kt-outer loop with one PSUM accumulator per output m-tile so
            # matmuls start as soon as each 1MB hsT k-slice lands (DMA
            # pipelining).  Head-pair 0 (qkvT m-tiles 0/2/4) and its V
            # transposes are emitted first so the attention phase can begin
            # while pair 1's projection still occupies the PE.
            with (
                tc.tile_pool(name="p1sb", bufs=1) as p1sb,
                tc.tile_pool(name="p1ps", bufs=6, space="PSUM") as p1ps,
            ):
                hsT_sb = p1sb.tile([P, KT_HS, S], F32R)     # 64 KB/part
                w_sb = p1sb.tile([P, KT_HS, 3 * HPC * HD], F32R)  # 24 KB/part
                hsT_r = hsT.rearrange("(t p) n -> p t n", p=P)
                w_r = wqkv.rearrange("(t p) n -> p t n", p=P)
                # Load order matters: the first QKV pass (nch=0) needs only
                # columns 0:512 of every k-slice, so stream (W_kt, hs_kt
                # nch-0 slice) pairs first, then the remaining n-chunks.
                # This cuts the DMA-paced startup gap from ~26us to ~7us.
                for kt in range(KT_HS):
                    nc.sync.dma_start(w_sb[:, kt, :], w_r[:, kt, :])
                    nc.sync.dma_start(hsT_sb[:, kt, 0:512],
                                      hsT_r[:, kt, 0:512])
                for nch in range(1, NSEQ):
                    for kt in range(KT_HS):
                        nc.sync.dma_start(
                            hsT_sb[:, kt, nch * 512:(nch + 1) * 512],
                            hsT_r[:, kt, nch * 512:(nch + 1) * 512])
                for pr in range(2):
                    mts = [0 + pr, 2 + pr, 4 + pr]
                    for nch in range(NSEQ):
                        ps_l = {mt: p1ps.tile([P, 512], F32, tag="qkv_ps",
                                              name=f"qkv_ps{pr}_{nch}_{mt}")
                                for mt in mts}
                        for kt in range(KT_HS):
                            for mt in mts:
                                nc.tensor.matmul(
                                    ps_l[mt][:],
                                    w_sb[:, kt, mt * P:(mt + 1) * P],
                                    hsT_sb[:, kt, nch * 512:(nch + 1) * 512],
                                    start=(kt == 0), stop=(kt == KT_HS - 1),
                                )
                        for mt in mts:
                            nc.vector.tensor_scalar_add(
                                qkvT[:, mt, nch * 512:(nch + 1) * 512],
                                ps_l[mt][:], bq_sb[:, mt:mt + 1])
                    # V transpose for this pair
                    for kt in range(KT_S):
                        tp = p1ps.tile([P, P], F32R, tag="vt", bufs=2,
                                       name=f"vt{pr}_{kt}")
                        nc.tensor.transpose(
                            tp[:], qkvT[:, 4 + pr, kt * P:(kt + 1) * P],
                            ident_f[:])
                        for hl in range(2):
                            nc.vector.tensor_copy(
                                v_sb[:, 2 * pr + hl, kt, 0:HD],
                                tp[:, hl * HD:(hl + 1) * HD])

            if dbg:
                nc.sync.dma_start(d_qkvT, qkvT[:].bitcast(F32))
                nc.sync.dma_start(d_v, v_sb[:].bitcast(F32))

            # ---------------- Phase 2: attention ----------------
            with (
                tc.tile_pool(name="mp", bufs=3) as mp,
                tc.tile_pool(name="ptp", bufs=int(os.environ.get("BK_PTP", "4"))) as ptp,
                tc.tile_pool(name="np_", bufs=2) as np_,
                tc.tile_pool(name="sps", bufs=2, space="PSUM") as sps,
                tc.tile_pool(name="cps", bufs=1, space="PSUM") as cps,
            ):
                if BK_MASK in ("psum", "hybrid"):
                    # Warm both s-slots: set every has_written bit with a
                    # throwaway matmul so later start=False accumulates onto
                    # DVE-written PSUM work (see dbg_hw.py).  Scrap reads
                    # keep DCE from dropping the warm-up matmuls.
                    scrap = np_.tile([P, 4], F32, name="scrap")
                    for w in range(2):
                        s_ps = sps.tile([P, QC], F32, tag="s",
                                        name=f"warm{w}")
                        for ch in range(QC // 512):
                            cs = slice(ch * 512, (ch + 1) * 512)
                            nc.tensor.matmul(
                                s_ps[:, cs], ident_f[:],
                                qkvT[:, 0, 0:512], start=True, stop=True)
                        nc.vector.tensor_copy(scrap[:, 2 * w:2 * w + 2],
                                              s_ps[:, 0:2])
                for pr in range(2):
                    for qc in range(NQC):
                        q0 = qc * QC
                        ctx_ps = [cps.tile([HD + 1, QC], F32, tag=f"ctx{hl}",
                                           name=f"ctx_ps{hl}")
                                  for hl in range(2)]
                        for ktg in range(KT_S // MB_KT):
                            mt_t = mp.tile([P, MB_KT, QC],
                                           F8 if BK_MASK in ("pe", "hybrid")
                                           else BF16,
                                           tag="mask")
                            nc.sync.dma_start(
                                mt_t[:],
                                maskT[ktg * MB_KT * P:(ktg + 1) * MB_KT * P,
                                      q0:q0 + QC].rearrange(
                                          "(g p) q -> p g q", p=P))
                            for kti in range(MB_KT):
                                kt = ktg * MB_KT + kti
                                s_ps = [sps.tile([P, QC], F32, tag="s",
                                                 name=f"s_ps{hl}")
                                        for hl in range(2)]
                                # Emission order: both heads' full-array
                                # mask-adds first, then the two k=64 QK
                                # matmuls adjacently — they target disjoint
                                # PE row-groups (partitions 0-63 / 64-127)
                                # and run concurrently on HW.
                                # (narrow-dtype mask-add must also come
                                # BEFORE the fp32r matmul of its group: a
                                # bf16/fp8 accumulate after an fp32r matmul
                                # corrupts the PSUM — PE weight-path hazard,
                                # see dbg_mask.)
                                for hl in range(2):
                                    if (BK_MASK == "psum"
                                            or (BK_MASK == "hybrid"
                                                and hl == 1)):
                                        nc.vector.tensor_copy(
                                            s_ps[hl][:], mt_t[:, kti, :])
                                    elif BK_MASK in ("pe", "hybrid"):
                                        for ch in range(QC // 512):
                                            cs = slice(ch * 512,
                                                       (ch + 1) * 512)
                                            nc.tensor.matmul(
                                                s_ps[hl][:, cs], ident_8[:],
                                                mt_t[:, kti, cs],
                                                start=True, stop=False)
                                for ch in range(QC // 512):
                                    cs = slice(ch * 512, (ch + 1) * 512)
                                    qs = slice(q0 + ch * 512,
                                               q0 + (ch + 1) * 512)
                                    for hl in range(2):
                                        rows = slice(hl * HD, (hl + 1) * HD)
                                        nc.tensor.matmul(
                                            s_ps[hl][:, cs],
                                            qkvT[rows, 2 + pr,
                                                 kt * P:(kt + 1) * P],
                                            qkvT[rows, 0 + pr, qs],
                                            start=(BK_MASK in
                                                   ("dve", "dvef")),
                                            stop=True,
                                            skip_group_check=(
                                                BK_MASK == "psum"
                                                or (BK_MASK == "hybrid"
                                                    and hl == 1)))
                                pts = []
                                for hl in range(2):
                                    pt = ptp.tile(
                                        [P, QC],
                                        BF16 if BK_MASK == "dve" else F32R,
                                        tag="pt")
                                    nc.scalar.activation(pt[:], s_ps[hl][:],
                                                         AF.Exp)
                                    if BK_MASK in ("dve", "dvef"):
                                        nc.vector.tensor_mul(
                                            pt[:], pt[:], mt_t[:, kti, :])
                                    pts.append(pt[:])
                                if dbg and pr == 0 and qc == 0 and kt == 0:
                                    nc.sync.dma_start(d_pt,
                                                      pts[0].bitcast(F32))
                                for hl in range(2):
                                    for ch in range(QC // 512):
                                        cs = slice(ch * 512, (ch + 1) * 512)
                                        nc.tensor.matmul(
                                            ctx_ps[hl][:, cs],
                                            v_sb[:, 2 * pr + hl, kt, :],
                                            pts[hl][:, cs],
                                            start=(kt == 0),
                                            stop=(kt == KT_S - 1))
                        # normalize: ctx_all[:, h, q0:q0+QC] = ctx / rowsum.
                        # rowsum sits at PSUM partition HD; reciprocal there,
                        # then broadcast across partitions with a k=1 matmul
                        # outer product (ones[1,HD].T @ rinv[1,QC]).
                        for hl in range(2):
                            h = 2 * pr + hl
                            rrow = np_.tile([HD + 1, QC], F32R, tag="rr")
                            # ACT has headroom; keep these copies off the
                            # pacing DVE
                            nc.scalar.copy(rrow[HD:HD + 1, :],
                                           ctx_ps[hl][HD:HD + 1, :])
                            # broadcast raw rowsum across partitions with a
                            # k=1 fp32 matmul, then reciprocal from PSUM.
                            # (reciprocal_approx_fast is broken on HW for
                            # single-partition APs; [64,N] tiles are fine.)
                            rb_ps = sps.tile([HD, QC], F32, tag="s")
                            if BK_MASK in ("psum", "hybrid"):
                                # start=True would clear has_written on this
                                # shared s-bank; zero it with DVE and
                                # accumulate instead.
                                nc.vector.memset(rb_ps[:], 0.0)
                            for ch in range(QC // 512):
                                cs = slice(ch * 512, (ch + 1) * 512)
                                nc.tensor.matmul(
                                    rb_ps[:, cs],
                                    ones_sb[HD:HD + 1, :],
                                    rrow[HD:HD + 1, cs],
                                    start=(BK_MASK not in ("psum", "hybrid")),
                                    stop=True,
                                    skip_group_check=(
                                        BK_MASK in ("psum", "hybrid")))
                            rbi = np_.tile([HD, QC], F32, tag="rbi")
                            nc.vector.reciprocal_approx_fast(rbi[:], rb_ps[:])
                            uh = np_.tile([HD, QC], F32, tag="uh")
                            nc.scalar.copy(uh[:], ctx_ps[hl][0:HD, :])
                            if dbg:
                                nc.sync.dma_start(d_u[:, h, q0:q0 + QC], uh[:])
                                nc.sync.dma_start(
                                    d_rr[4 * pr + 2 * qc + hl:
                                         4 * pr + 2 * qc + hl + 1, :],
                                    rrow[HD:HD + 1, :].bitcast(F32))
                            # head pairs stack into [128, S] dense lhsT
                            # tiles; odd heads go to partitions 64-127 via a
                            # small SBUF->SBUF DMA (engines cannot cross
                            # partitions).
                            if hl == 0:
                                nc.vector.tensor_mul(
                                    ctx_pair[0:HD, pr, q0:q0 + QC],
                                    uh[:], rbi[:])
                            else:
                                stg = np_.tile([HD, QC], F32R, tag="stg")
                                nc.vector.tensor_mul(stg[:], uh[:], rbi[:])
                                nc.sync.dma_start(
                                    ctx_pair[HD:P, pr, q0:q0 + QC], stg[:])

            if dbg:
                nc.sync.dma_start(d_ctx, ctx_pair[:].bitcast(F32))

            # ---------------- Phase 3: dense partial ----------------
            with (
                tc.tile_pool(name="yp", bufs=3) as yp,
                tc.tile_pool(name="dps", bufs=3, space="PSUM") as dps,
            ):
                for mt in range(S // P):
                    yt = yp.tile([P, D], F32, tag="y")
                    for nch in range(D // 512):
                        ps = dps.tile([P, 512], F32, tag="d")
                        for pr in range(2):
                            nc.tensor.matmul(
                                ps[:],
                                ctx_pair[:, pr, mt * P:(mt + 1) * P],
                                wd_sb[:, pr, nch * 512:(nch + 1) * 512],
                                start=(pr == 0), stop=(pr == 1))
                        # split tail copies across ACT and the
                        # tail-idle DVE
                        if nch == 0:
                            nc.scalar.copy(yt[:, nch * 512:(nch + 1) * 512],
                                           ps[:])
                        else:
                            nc.vector.tensor_copy(
                                yt[:, nch * 512:(nch + 1) * 512], ps[:])
                    nc.sync.dma_start(y[mt * P:(mt + 1) * P, :], yt[:])

    nc.compile()
    return nc


_NC = None


def get_program():
    global _NC
    if _NC is None:
        _NC = build_program()
    return _NC


def make_in_maps(hidden_states, attention_mask, W_qkv, b_qkv, W_dense, b_dense):
    hs = np.asarray(hidden_states, np.float32)
    mask = np.asarray(attention_mask)
    W_qkv = np.asarray(W_qkv, np.float32)
    b_qkv = np.asarray(b_qkv, np.float32)
    W_dense = np.asarray(W_dense, np.float32)

    hsT = [np.ascontiguousarray(hs[b].T) for b in range(B)]
    maskT_add = []
    for b in range(B):
        if BK_MASK in ("pe", "hybrid"):
            m = np.where(mask[b, 0], 0.0, MASK_NEG).astype(np.float32).T
            maskT_add.append(
                np.ascontiguousarray(m).astype(ml_dtypes.float8_e5m2))
        elif BK_MASK == "psum":
            m = np.where(mask[b, 0], 0.0, MASK_NEG).astype(np.float32).T
            maskT_add.append(np.ascontiguousarray(m).astype(ml_dtypes.bfloat16))
        else:  # dve / dvef: multiplicative
            m = np.where(mask[b, 0], 1.0, 0.0).astype(np.float32).T
            maskT_add.append(np.ascontiguousarray(m).astype(ml_dtypes.bfloat16))

    Wq, Wk, Wv = W_qkv[:, :D], W_qkv[:, D:2 * D], W_qkv[:, 2 * D:]
    bq, bk, bv = b_qkv[:D], b_qkv[D:2 * D], b_qkv[2 * D:]

    in_maps = []
    for c in range(NCORES):
        b = c // (NCORES // B)
        h0 = HPC * (c % (NCORES // B))
        cols = slice(h0 * HD, (h0 + HPC) * HD)
        wq_c = Wq[:, cols] / SCALE
        wk_c = Wk[:, cols]
        wv_c = Wv[:, cols]
        wqkv_c = np.ascontiguousarray(
            np.concatenate([wq_c, wk_c, wv_c], axis=1), dtype=np.float32)
        bqkv_c = np.concatenate(
            [bq[cols] / SCALE, bk[cols], bv[cols]]).astype(np.float32)
        wd_c = np.ascontiguousarray(W_dense[cols, :], dtype=np.float32)
        in_maps.append({
            "hsT": hsT[b],
            "wqkv": wqkv_c,
            "bqkv": bqkv_c,
            "maskT": maskT_add[b],
            "wd": wd_c,
        })
    return in_maps


def kernel(hidden_states, attention_mask, W_qkv, b_qkv, W_dense, b_dense,
           **run_kwargs):
    nc = get_program()
    in_maps = make_in_maps(hidden_states, attention_mask, W_qkv, b_qkv,
                           W_dense, b_dense)
    res = bass_utils.run_bass_kernel_spmd(
        nc, in_maps, core_ids=list(range(NCORES)), **run_kwargs)
    out = np.zeros((B, S, D), np.float32)
    gpb = NCORES // B
    for c in range(NCORES):
        out[c // gpb] += res.results[c]["y"]
    out += np.asarray(b_dense, np.float32)
    if run_kwargs:
        kernel.last_results = res
    return out



# revision 4
# speedup vs baseline: 1.1225x; 1.1225x over previous
"""Trainium2 Bass kernel for ExpBertSelfAttention (B=2, S=2048, D=1024, H=16).

Sharding: 8 cores; core c handles batch b=c//4 and 4 consecutive heads
4*(c%4)..4*(c%4)+3 (data-parallel on B, tensor-parallel on heads).  The dense
output projection is row-parallel, so each core returns a partial [S, D] sum;
the host adds the 4 partials per batch plus b_dense.

v2 design (vs the v1 baseline): the engine-limiting work in the cost model is
elementwise (ACT exp + DVE mask-mul), so the kernel is restructured so every
non-exp pass is cheap:

  - Q,K projected TRANSPOSED (qkT [ch, seq], f32r) for the QK matmul;
    V projected in NATURAL layout ([seq, ch], bf16) straight from hsT
    (lhsT = hsT k-slice), with an augmented Wv that leaves a zero column
    per head; a host-built vbias tile then adds b_v and writes 1.0 into
    those columns, giving each head a "ones" column for free.
  - scores computed transposed: s[kseq_tile, q] (lhsT = kT slice).
  - exp on ACT -> bf16 probs; mask applied multiplicatively on DVE in
    bf16 (2x_1p mode; exact 0/1 mask).
  - PV with swapped operands: lhsT = prob tile [kseq, q_tile(128)]
    (stationary; LDWEIGHTS), rhs = V [kseq, 65] streaming only 65 cols.
    ctx lands NON-transposed [q, hd] in PSUM with the softmax rowsum in
    col 64 -> normalization is a per-partition reciprocal +
    tensor_scalar_mul (no broadcast matmuls, no big copies).
  - normalized bf16 ctx [q, hd] is transposed back for the dense matmul
    with DMA-transpose (XBAR, bf16) -- no PE/PSUM involvement.
  - dense: lhsT = ctxT [hhd, q_tile] bf16, rhs = wd bf16, PSUM f32 out,
    evacuated by DVE/ACT copies and DMA'd out.

Precision: scores path in f32r; probs/V/ctx/wd in bf16 (measured end-to-end
rel err ~3e-3 vs the 2e-2 gate).
"""

import os
import sys

for _p in ("/opt/trn_rl_repo", "/root/.axon_site/_ro/trn_rl_repo"):
    if os.path.isdir(_p) and _p not in sys.path:
        sys.path.insert(0, _p)

import numpy as np
import ml_dtypes

import concourse.bass as bass
import concourse.tile as tile
from concourse import bacc, mybir
from concourse import bass_utils

B, S, D, H = 2, 2048, 1024, 16
HD = D // H  # 64
SCALE = float(np.sqrt(HD).astype(np.float32))
NCORES = 8
HPC = H // (NCORES // B)  # heads per core = 4
P = 128
F32 = mybir.dt.float32
F32R = mybir.dt.float32r
BF16 = mybir.dt.bfloat16
AF = mybir.ActivationFunctionType

KT_HS = D // P        # 8 contraction tiles for projections
KT_S = S // P         # 16 key tiles for attention
QC = 1024             # q chunk for scores/probs
NQC = S // QC         # 2
NQT = QC // P         # 8 q-tiles of 128 per chunk
ST = S // P           # 16 seq tiles
VW = HPC * (HD + 1)   # 260: V columns incl per-head ones column


def build_program():
    nc = bacc.Bacc("TRN2", target_bir_lowering=False, debug=False,
                   num_devices=NCORES)

    hsT = nc.dram_tensor("hsT", [D, S], F32R, kind="ExternalInput").ap()
    # wqk columns: [Q01 | K01 | Q23 | K23], 128 each; Q pre-divided by SCALE
    wqk = nc.dram_tensor("wqk", [D, 4 * P], F32R, kind="ExternalInput").ap()
    bqk = nc.dram_tensor("bqk", [P, 4], F32, kind="ExternalInput").ap()
    # wv columns: h*65+(0:64) = Wv of head h; col h*65+64 = 0
    wv = nc.dram_tensor("wv", [D, VW], F32R, kind="ExternalInput").ap()
    # vbias: b_v replicated across partitions; 1.0 at the ones columns
    vbias = nc.dram_tensor("vbias", [P, VW], F32, kind="ExternalInput").ap()
    maskT = nc.dram_tensor("maskT", [S, S], BF16, kind="ExternalInput").ap()
    wd = nc.dram_tensor("wd", [HPC * HD, D], BF16, kind="ExternalInput").ap()
    y = nc.dram_tensor("y", [S, D], F32, kind="ExternalOutput").ap()

    with tile.TileContext(nc) as tc:
        with (
            tc.tile_pool(name="persist", bufs=1) as persist,
            tc.tile_pool(name="mp", bufs=5) as mp,        # mask tiles (4 alive per qc + 1 prefetch)
            tc.tile_pool(name="ptp", bufs=3) as ptp,      # prob tiles
            tc.tile_pool(name="np_", bufs=2) as np_,      # small norm tiles
            tc.tile_pool(name="ysb", bufs=2) as ysb,      # y staging
            tc.tile_pool(name="sps", bufs=2, space="PSUM") as sps,
            tc.tile_pool(name="cps", bufs=1, space="PSUM") as cps,
            tc.tile_pool(name="aux", bufs=2, space="PSUM") as aux,
        ):
            hsT_sb = persist.tile([P, KT_HS, S], F32R)        # 64 KB/part
            wqk_sb = persist.tile([P, KT_HS, 4 * P], F32R)    # 16 KB/part
            wv_sb = persist.tile([P, KT_HS, VW], F32R)        # 8.3 KB/part
            qkT = persist.tile([P, 4, S], F32R)               # 32 KB/part
            v_sb = persist.tile([P, KT_S, VW], BF16)          # 8.3 KB/part
            ctx_sb = persist.tile([P, 2, NQT, P], BF16)       # 4 KB/part
            ctxT = persist.tile([P, 2, S], BF16)              # 8 KB/part
            wd_sb = persist.tile([P, 2, D], BF16)             # 4 KB/part
            bqk_sb = persist.tile([P, 4], F32)
            vbias_sb = persist.tile([P, VW], F32)

            nc.sync.dma_start(wd_sb[:], wd.rearrange("(t p) n -> p t n", p=P))
            nc.sync.dma_start(bqk_sb[:], bqk)
            nc.sync.dma_start(vbias_sb[:], vbias)
            hsT_r = hsT.rearrange("(t p) n -> p t n", p=P)
            w_r = wqk.rearrange("(t p) n -> p t n", p=P)
            wv_r = wv.rearrange("(t p) n -> p t n", p=P)
            # stream (W k-slice, hs k-slice) pairs so the first proj chains
            # can start as soon as their inputs land
            for kt in range(KT_HS):
                nc.sync.dma_start(wqk_sb[:, kt, :], w_r[:, kt, :])
                nc.sync.dma_start(wv_sb[:, kt, :], wv_r[:, kt, :])
                nc.sync.dma_start(hsT_sb[:, kt, :], hsT_r[:, kt, :])

            # ---- projection chain emitters (PE filler units) ----
            def qk_chunk(mt, ch):
                """One qkT output chunk: accumulate 8 k-tiles, bias-add out."""
                ps = aux.tile([P, 512], F32, tag="aux")
                for kt in range(KT_HS):
                    nc.tensor.matmul(
                        ps[:], wqk_sb[:, kt, mt * P:(mt + 1) * P],
                        hsT_sb[:, kt, ch * 512:(ch + 1) * 512],
                        start=(kt == 0), stop=(kt == KT_HS - 1))
                nc.vector.tensor_scalar_add(
                    qkT[:, mt, ch * 512:(ch + 1) * 512], ps[:],
                    bqk_sb[:, mt:mt + 1])

            def v_chunk(st):
                """V natural [seq_tile, 260] incl. bias + ones columns."""
                ps = aux.tile([P, 512], F32, tag="aux")
                for kt in range(KT_HS):
                    nc.tensor.matmul(
                        ps[:, 0:VW], hsT_sb[:, kt, st * P:(st + 1) * P],
                        wv_sb[:, kt, :],
                        start=(kt == 0), stop=(kt == KT_HS - 1))
                nc.vector.tensor_add(v_sb[:, st, :], ps[:, 0:VW], vbias_sb[:])

            def dense_qt(qc, qt):
                """Dense partial for one q-tile of 128 rows."""
                yt = ysb.tile([P, D], F32, tag="y")
                q0 = qc * QC + qt * P
                for nchh in range(2):
                    ps = aux.tile([P, 512], F32, tag="aux")
                    for pr in range(2):
                        nc.tensor.matmul(
                            ps[:], ctxT[:, pr, q0:q0 + P],
                            wd_sb[:, pr, nchh * 512:(nchh + 1) * 512],
                            start=(pr == 0), stop=(pr == 1))
                    if nchh == 0:
                        nc.scalar.copy(yt[:, nchh * 512:(nchh + 1) * 512],
                                       ps[:])
                    else:
                        nc.vector.tensor_copy(
                            yt[:, nchh * 512:(nchh + 1) * 512], ps[:])
                nc.sync.dma_start(y[q0:q0 + P, :], yt[:])

            # filler queue: each entry emits one PE chain (~8 matmuls)
            filler = []
            for st in range(2, ST):
                filler.append(lambda st=st: v_chunk(st))
            for mt in (2, 3):  # Q23, K23
                for ch in range(4):
                    filler.append(lambda mt=mt, ch=ch: qk_chunk(mt, ch))

            # ---- warm-up: pair-0 projections + first V tiles ----
            for mt in (0, 1):
                for ch in range(4):
                    qk_chunk(mt, ch)
            v_chunk(0)
            v_chunk(1)

            # ---- attention ----
            # per (qc): heads 0..3; per head: 16 key tiles; after a pair's
            # two heads finish, DMA-transpose their ctx into ctxT; after
            # pair 1, the dense for this qc is queued as filler into the
            # next qc's attention (or run at the end for the last qc).
            for qc in range(NQC):
                q0 = qc * QC
                # mask tiles for this q chunk, 4 key-tiles per DMA
                mt_t = [None] * 4
                for ktg in range(4):
                    t = mp.tile([P, 4, QC], BF16, tag="mask")
                    nc.sync.dma_start(
                        t[:],
                        maskT[ktg * 4 * P:(ktg + 1) * 4 * P,
                              q0:q0 + QC].rearrange("(g p) q -> p g q", p=P))
                    mt_t[ktg] = t
                for h in range(HPC):
                    pr, hl = divmod(h, 2)
                    rows = slice(hl * HD, (hl + 1) * HD)
                    ctx_b = [cps.tile([P, 512], F32, tag=f"ctx{i}",
                                      name=f"ctx{i}_{h}_{qc}")
                             for i in range(2)]
                    for kt in range(KT_S):
                        s_ps = sps.tile([P, QC], F32, tag="s")
                        for ch in range(QC // 512):
                            cs = slice(ch * 512, (ch + 1) * 512)
                            nc.tensor.matmul(
                                s_ps[:, cs],
                                qkT[rows, 2 * pr + 1, kt * P:(kt + 1) * P],
                                qkT[rows, 2 * pr, q0 + ch * 512:
                                    q0 + (ch + 1) * 512],
                                start=True, stop=True)
                        pt = ptp.tile([P, QC], BF16, tag="pt")
                        nc.scalar.activation(pt[:], s_ps[:], AF.Exp)
                        nc.vector.tensor_mul(pt[:], pt[:],
                                             mt_t[kt // 4][:, kt % 4, :])
                        for qt in range(NQT):
                            bank, off = divmod(qt, 4)
                            nc.tensor.matmul(
                                ctx_b[bank][:, off * P:off * P + HD + 1],
                                pt[:, qt * P:(qt + 1) * P],
                                v_sb[:, kt, h * (HD + 1):(h + 1) * (HD + 1)],
                                start=(kt == 0), stop=(kt == KT_S - 1))
                        # PE filler: keep the tensor engine fed while ACT
                        # paces the loop
                        if filler and kt % 2 == 1:
                            filler.pop(0)()
                    # normalize: rowsum sits at col off*128+64 of each bank
                    rinv = np_.tile([P, NQT], F32, tag="rinv")
                    for bank in range(2):
                        nc.vector.reciprocal(
                            rinv[:, 4 * bank:4 * bank + 4].rearrange(
                                "p (a b) -> p a b", b=1),
                            ctx_b[bank][:].rearrange(
                                "p (a b) -> p a b", b=P)[:, :, HD:HD + 1])
                    for qt in range(NQT):
                        bank, off = divmod(qt, 4)
                        nc.vector.tensor_scalar_mul(
                            ctx_sb[:, pr, qt, rows],
                            ctx_b[bank][:, off * P:off * P + HD],
                            rinv[:, qt:qt + 1])
                    if hl == 1:
                        # pair done at this qc: transpose ctx into ctxT
                        for qt in range(NQT):
                            nc.sync.dma_start(
                                ctxT[:, pr, q0 + qt * P:q0 + (qt + 1) * P],
                                ctx_sb[:, pr, qt, :], transpose=True)
                if qc == 0:
                    # queue qc0's dense as filler for qc1's attention
                    for qt in range(NQT):
                        filler.append(lambda qt=qt: dense_qt(0, qt))
            while filler:
                filler.pop(0)()
            for qt in range(NQT):
                dense_qt(NQC - 1, qt)

    nc.compile()
    return nc


_NC = None


def get_program():
    global _NC
    if _NC is None:
        _NC = build_program()
    return _NC


def make_in_maps(hidden_states, attention_mask, W_qkv, b_qkv, W_dense, b_dense):
    hs = np.asarray(hidden_states, np.float32)
    mask = np.asarray(attention_mask)
    W_qkv = np.asarray(W_qkv, np.float32)
    b_qkv = np.asarray(b_qkv, np.float32)
    W_dense = np.asarray(W_dense, np.float32)

    hsT = [np.ascontiguousarray(hs[b].T) for b in range(B)]
    maskT_m = [np.ascontiguousarray(
        np.where(mask[b, 0], 1.0, 0.0).astype(np.float32).T
    ).astype(ml_dtypes.bfloat16) for b in range(B)]

    Wq, Wk, Wv = W_qkv[:, :D], W_qkv[:, D:2 * D], W_qkv[:, 2 * D:]
    bq, bk, bv = b_qkv[:D], b_qkv[D:2 * D], b_qkv[2 * D:]

    in_maps = []
    for c in range(NCORES):
        b = c // (NCORES // B)
        h0 = HPC * (c % (NCORES // B))
        # wqk: [Q01/SCALE | K01 | Q23/SCALE | K23]
        cols01 = slice(h0 * HD, (h0 + 2) * HD)
        cols23 = slice((h0 + 2) * HD, (h0 + 4) * HD)
        wqk_c = np.concatenate(
            [Wq[:, cols01] / SCALE, Wk[:, cols01],
             Wq[:, cols23] / SCALE, Wk[:, cols23]], axis=1)
        bqk_c = np.stack(
            [bq[cols01] / SCALE, bk[cols01],
             bq[cols23] / SCALE, bk[cols23]], axis=1)
        # wv augmented with zero ones-columns; vbias carries b_v and the 1.0s
        wv_c = np.zeros((D, VW), np.float32)
        vb_c = np.zeros((VW,), np.float32)
        for hh in range(HPC):
            csl = slice((h0 + hh) * HD, (h0 + hh + 1) * HD)
            wv_c[:, hh * (HD + 1):hh * (HD + 1) + HD] = Wv[:, csl]
            vb_c[hh * (HD + 1):hh * (HD + 1) + HD] = bv[csl]
            vb_c[hh * (HD + 1) + HD] = 1.0
        vbias_c = np.broadcast_to(vb_c, (P, VW))
        wd_c = np.ascontiguousarray(
            W_dense[h0 * HD:(h0 + HPC) * HD, :]).astype(ml_dtypes.bfloat16)
        in_maps.append({
            "hsT": hsT[b],
            "wqk": np.ascontiguousarray(wqk_c),
            "bqk": np.ascontiguousarray(bqk_c),
            "wv": np.ascontiguousarray(wv_c),
            "vbias": np.ascontiguousarray(vbias_c),
            "maskT": maskT_m[b],
            "wd": wd_c,
        })
    return in_maps


def kernel(hidden_states, attention_mask, W_qkv, b_qkv, W_dense, b_dense,
           **run_kwargs):
    nc = get_program()
    in_maps = make_in_maps(hidden_states, attention_mask, W_qkv, b_qkv,
                           W_dense, b_dense)
    res = bass_utils.run_bass_kernel_spmd(
        nc, in_maps, core_ids=list(range(NCORES)), **run_kwargs)
    out = np.zeros((B, S, D), np.float32)
    gpb = NCORES // B
    for c in range(NCORES):
        out[c // gpb] += res.results[c]["y"]
    out += np.asarray(b_dense, np.float32)
    if run_kwargs:
        kernel.last_results = res
    return out
